# revision 7
# baseline (speedup 1.0000x reference)
"""Cross-attention kernel for Trainium2, 8-core SPMD.

Problem (hardcoded shapes): B=4, N=4096, S=512, DIM=1024, H=16, D=64.
Sharding: data-parallel over B (4) x tensor-parallel over head-groups (2).
Each core computes 8 heads for one batch; host sums the two head-group
partial projection outputs per batch.

Per-core math (g = head group, b = batch):
  QT = qw_g.T @ x_b.T          [512, 4096]   (q-features on partitions)
  KT = kw_g.T @ ctx_b.T        [512, 512]
  V  = ctx_b @ vw_g            [512, 512]    (s on partitions)
  per head h (64 features), per 512-token chunk:
    S.T  = KT_h.T-slice @ QT_h [s=512, n]    scores transposed
    E    = exp(S.T * 0.125)                  masked V rows are zeroed and the
                                             denominator column of V holds the
                                             0/1 mask, so exp needs no bias
    O'   = [V_h*m | m].T @ E   [65, n]       row 64 = masked softmax denominator
    O.T  = O'[0:64] * (1/O'[64]) broadcast
  out_partial = O.T-as-lhsT @ pw_g  (host adds proj bias + partner partial)

v2 schedule: a uniform software pipeline over half-wave steps
t = 8*chunk + 2*wavepair + head (64 steps of ~3.6us PE work each):
  step t emits: scores(t) -> AV(t-2) + lane-local reciprocal -> PE
  broadcast of 1/z for step t-3 (tiny K=1 matmul instead of the ~1us
  gpsimd partition_broadcast) -> DVE copy+multiply normalize.
  QT tiles of the next chunk go at phases 0-3, out-projection tiles of
  the previous chunk at phases 4-7, so PE work is even per step and AV
  always trails its exp by two full steps (Act engine never stalls PE).
Act engine runs only exps + qt/kt bias evictions; all other evictions
and elementwise work are on DVE. Partial outputs are written fp16.
"""
import os
import numpy as np

P = 128
B, N, S, DIM = 4, 4096, 512, 1024
HEADS, D = 16, 64
HG = 8               # heads per core
GF = HG * D          # 512 features per head-group
NCHUNK = 512
NCH = N // NCHUNK    # 8 chunks
KT_TILES = DIM // P  # 8 contraction tiles for projections
SCALE = D ** -0.5

LAST_RESULTS = None
_CACHED_NC = None


def _build():
    import concourse.mybir as mybir
    import concourse.tile as tile
    from concourse import bacc

    f32 = mybir.dt.float32
    f16 = mybir.dt.float16  # matmul operand dtype: fp16 streams 1 col/cycle

    nc = bacc.Bacc("TRN2", target_bir_lowering=False, debug=False)

    xT = nc.dram_tensor("xT", [DIM, N], f16, kind="ExternalInput")
    ctxT = nc.dram_tensor("ctxT", [DIM, S], f16, kind="ExternalInput")
    qw = nc.dram_tensor("qw", [DIM, GF], f16, kind="ExternalInput")
    kw = nc.dram_tensor("kw", [DIM, GF], f16, kind="ExternalInput")
    vw = nc.dram_tensor("vw", [DIM, GF], f16, kind="ExternalInput")
    pw = nc.dram_tensor("pw", [GF, DIM], f16, kind="ExternalInput")
    qb = nc.dram_tensor("qb", [P, GF // P], f32, kind="ExternalInput")
    kb = nc.dram_tensor("kb", [P, GF // P], f32, kind="ExternalInput")
    vbm = nc.dram_tensor("vbm", [P, S // P, GF], f16, kind="ExternalInput")
    m01 = nc.dram_tensor("m01", [P, S // P], f32, kind="ExternalInput")
    o = nc.dram_tensor("o", [N, DIM], f16, kind="ExternalOutput")

    JQ = GF // P        # 4 q-feature tiles
    ST = S // P         # 4 s tiles
    TOT = NCH * 8       # 64 half-wave steps

    with tile.TileContext(nc) as tc:
        with (
            tc.tile_pool(name="const", bufs=1) as cpool,
            tc.tile_pool(name="kv", bufs=1) as kvpool,
            tc.tile_pool(name="qt", bufs=2) as qtpool,
            tc.tile_pool(name="xq", bufs=2) as xqpool,
            tc.tile_pool(name="e", bufs=4) as epool,
            tc.tile_pool(name="ot", bufs=2) as otpool,
            tc.tile_pool(name="stgf", bufs=2) as stgfpool,
            tc.tile_pool(name="stgh", bufs=3) as stghpool,
            tc.tile_pool(name="rb", bufs=2) as rbpool,
            tc.tile_pool(name="ost", bufs=2) as ostpool,
            tc.tile_pool(name="ps_p", bufs=2, space="PSUM") as ps_p,
            tc.tile_pool(name="ps_s", bufs=2, space="PSUM") as ps_s,
            tc.tile_pool(name="ps_o", bufs=2, space="PSUM") as ps_o,
        ):
            # ---- SBUF residents ----
            qw_sb = cpool.tile([P, KT_TILES, GF], f16)
            kw_sb = cpool.tile([P, KT_TILES, GF], f16)
            vw_sb = cpool.tile([P, KT_TILES, GF], f16)
            pw_sb = cpool.tile([P, GF // P, DIM], f16)
            ctx_sb = xqpool.tile([P, KT_TILES, S], f16, tag="xq")
            qb_sb = cpool.tile([P, JQ], f32)
            kb_sb = cpool.tile([P, JQ], f32)
            vbm_sb = cpool.tile([P, ST, GF], f16)
            m01_sb = cpool.tile([P, ST], f32)
            ones_sb = cpool.tile([P, D], f16)
            nc.vector.memset(ones_sb[:], 1.0)
            # persistent reciprocal staging: rows 1..127 stay 1.0 so the
            # full-tile reciprocal_approx_fast (custom-DVE ucode wants
            # offset-0 full-partition APs) is well defined
            stage_all = cpool.tile([P, 2, NCHUNK], f32)
            nc.vector.memset(stage_all[:], 1.0)

            # ---- preamble DMAs in dependency order ----
            # kw/ctx split per k-tile so the KT k-loop starts ~10us in.
            kw_r = kw.rearrange("(ko ki) m -> ki ko m", ki=P)
            ctx_r = ctxT.rearrange("(ko ki) s -> ki ko s", ki=P)
            x_r = xT.rearrange("(ko ki) n -> ki ko n", ki=P)
            nc.sync.dma_start(kb_sb[:], kb[:])
            nc.sync.dma_start(qb_sb[:], qb[:])
            nc.sync.dma_start(m01_sb[:], m01[:])
            for k in range(KT_TILES):
                nc.sync.dma_start(kw_sb[:, k, :], kw_r[:, k, :])
                nc.sync.dma_start(ctx_sb[:, k, :], ctx_r[:, k, :])
            nc.sync.dma_start(vw_sb[:], vw.rearrange("(ko ki) m -> ki ko m", ki=P))
            nc.sync.dma_start(vbm_sb[:], vbm[:])
            nc.sync.dma_start(qw_sb[:], qw.rearrange("(ko ki) m -> ki ko m", ki=P))

            XQ = {}

            def issue_xq(c):
                xq = xqpool.tile([P, KT_TILES, NCHUNK], f16, tag="xq", name="xq")
                nc.sync.dma_start(xq[:], x_r[:, :, c * NCHUNK:(c + 1) * NCHUNK])
                XQ[c] = xq

            issue_xq(0)
            nc.sync.dma_start(pw_sb[:], pw.rearrange("(ko ki) m -> ki ko m", ki=P))

            # ---- preamble compute: KT, V, QT(chunk 0) ----
            kt_sb = kvpool.tile([P, JQ, S], f16)
            for jk in range(JQ):
                ps = ps_p.tile([P, S], f32, tag="proj_ps", name="ps")
                for k in range(KT_TILES):
                    nc.tensor.matmul(
                        ps[:], kw_sb[:, k, jk * P:(jk + 1) * P], ctx_sb[:, k, :],
                        start=(k == 0), stop=(k == KT_TILES - 1))
                nc.scalar.activation(kt_sb[:, jk, :], ps[:],
                                     mybir.ActivationFunctionType.Identity,
                                     bias=kb_sb[:, jk:jk + 1])

            # V rows for masked s are zeroed; col D holds the 0/1 mask so
            # masked positions drop out of numerator and denominator.
            v_sb = kvpool.tile([P, ST, HG, D + 1], f16)
            for st in range(ST):
                ps = ps_p.tile([P, GF], f32, tag="proj_ps", name="ps")
                for k in range(KT_TILES):
                    nc.tensor.matmul(
                        ps[:], ctx_sb[:, k, st * P:(st + 1) * P], vw_sb[:, k, :],
                        start=(k == 0), stop=(k == KT_TILES - 1))
                nc.vector.scalar_tensor_tensor(
                    v_sb[:, st, :, 0:D],
                    ps.rearrange("p (h d) -> p h d", h=HG),
                    m01_sb[:, st:st + 1],
                    vbm_sb[:, st, :].rearrange("p (h d) -> p h d", h=HG),
                    mybir.AluOpType.mult, mybir.AluOpType.add)
                nc.scalar.activation(
                    v_sb[:, st, :, D], vbm_sb[:, st, 0:HG],
                    mybir.ActivationFunctionType.Identity,
                    bias=m01_sb[:, st:st + 1], scale=0.0)

            QT = {}

            def emit_qt_tile(c, jq):
                if jq == 0:
                    QT[c] = qtpool.tile([P, JQ, NCHUNK], f16, tag="qt", name="qt")
                qt = QT[c]
                xq = XQ[c]
                ps = ps_p.tile([P, NCHUNK], f32, tag="proj_ps", name="ps")
                for k in range(KT_TILES):
                    nc.tensor.matmul(
                        ps[:], qw_sb[:, k, jq * P:(jq + 1) * P], xq[:, k, :],
                        start=(k == 0), stop=(k == KT_TILES - 1))
                nc.scalar.activation(qt[:, jq, :], ps[:],
                                     mybir.ActivationFunctionType.Identity,
                                     bias=qb_sb[:, jq:jq + 1])

            for jq in range(JQ):
                emit_qt_tile(0, jq)
            issue_xq(1)

            # ---- pipeline stage emitters (step t = 8c + 2w + hh) ----
            E, OPS, STGH, OT = {}, {}, {}, {}

            def emit_scores(t):
                c, w, hh = t // 8, (t % 8) // 2, t % 2
                hb = 64 * hh
                qt_h = QT[c][hb:hb + 64, w, :]
                e = epool.tile([P, ST, NCHUNK], f16, tag="e", name="e")
                for stp in range(ST // 2):
                    sps = ps_s.tile([P, 2, NCHUNK], f32, tag="s_ps", name="sps")
                    for i in range(2):
                        st = 2 * stp + i
                        nc.tensor.matmul(
                            sps[:, i, :],
                            kt_sb[hb:hb + 64, w, st * P:(st + 1) * P],
                            qt_h, start=True, stop=True)
                    nc.scalar.activation(
                        e[:, 2 * stp:2 * stp + 2, :], sps[:],
                        mybir.ActivationFunctionType.Exp, scale=SCALE)
                E[t] = e

            def emit_av(t):
                c, w, hh = t // 8, (t % 8) // 2, t % 2
                h = 2 * w + hh
                e = E.pop(t)
                ops = ps_o.tile([D + 1, NCHUNK], f32, tag="o_ps", name="ops")
                for st in range(ST):
                    nc.tensor.matmul(
                        ops[:], v_sb[:, st, h, :], e[:, st, :],
                        start=(st == 0), stop=(st == ST - 1))
                # 1/z: bounce the denominator row to partition 0 of the
                # 1.0-filled stage tile, then full-tile approx reciprocal
                # (the custom-DVE op misreads narrow partition-64 APs on hw)
                slot = t % 2
                rcp = stgfpool.tile([P, NCHUNK], f32, tag="stgf", name="rcp")
                stgh = stghpool.tile([P, NCHUNK], f16, tag="stgh", name="stgh")
                nc.vector.tensor_copy(stage_all[0:1, slot, :], ops[D:D + 1, :])
                nc.vector.reciprocal_approx_fast(rcp[:], stage_all[:, slot, :])
                nc.vector.tensor_copy(stgh[0:1, :], rcp[0:1, :])
                OPS[t] = ops
                STGH[t] = stgh

            def emit_bcast_mul(t):
                c, w, hh = t // 8, (t % 8) // 2, t % 2
                hb = 64 * hh
                ops = OPS.pop(t)
                stgh = STGH.pop(t)
                rb = ps_p.tile([D, NCHUNK], f32, tag="proj_ps", name="rb")
                nc.tensor.matmul(rb[:], ones_sb[0:1, 0:D],
                                 stgh[0:1, :], start=True, stop=True)
                rbs = rbpool.tile([D, NCHUNK], f16, tag="rbs", name="rbs")
                nc.vector.tensor_copy(rbs[:], rb[:])
                nc.vector.tensor_mul(OT[c][hb:hb + 64, w, :], ops[0:D, :],
                                     rbs[:])

            OSTG = {}

            def emit_proj_half(c, tp):
                # one psum tile: tp = 2*ns + fh; DMA the row block after fh=1
                ns, fh = tp // 2, tp % 2
                ot = OT[c]
                if fh == 0:
                    OSTG[c * 4 + ns] = ostpool.tile([P, DIM], f16,
                                                    tag="ostage", name="ostage")
                ostage = OSTG[c * 4 + ns]
                ps = ps_p.tile([P, DIM // 2], f32, tag="proj_ps", name="ps")
                for j in range(JQ):
                    nc.tensor.matmul(
                        ps[:],
                        ot[:, j, ns * P:(ns + 1) * P],
                        pw_sb[:, j, fh * 512:(fh + 1) * 512],
                        start=(j == 0), stop=(j == JQ - 1))
                nc.scalar.copy(ostage[:, fh * 512:(fh + 1) * 512], ps[:])
                if fh == 1:
                    del OSTG[c * 4 + ns]
                    nc.sync.dma_start(
                        o[c * NCHUNK + ns * P: c * NCHUNK + (ns + 1) * P, :],
                        ostage[:])

            # ---- main uniform-step loop ----
            for t in range(TOT):
                c, ph = t // 8, t % 8
                if ph == 0:
                    OT[c] = otpool.tile([P, JQ, NCHUNK], f16, tag="ot", name="ot")
                emit_scores(t)
                if t >= 2:
                    emit_av(t - 2)
                if t >= 3:
                    emit_bcast_mul(t - 3)
                if ph % 2 == 1 and c + 1 < NCH:
                    emit_qt_tile(c + 1, ph // 2)
                if ph == 0 and c + 2 < NCH:
                    issue_xq(c + 2)
                # ot(c-1)'s last normalize mul is emitted at ph2 (lag-3
                # pipeline), so proj tiles of chunk c-1 start at ph2
                if c >= 1:
                    for tp in {2: [0, 1], 3: [2, 3], 4: [4], 5: [5],
                               6: [6], 7: [7]}.get(ph, []):
                        emit_proj_half(c - 1, tp)

            # ---- drain ----
            emit_av(TOT - 2)
            emit_bcast_mul(TOT - 3)
            emit_av(TOT - 1)
            emit_bcast_mul(TOT - 2)
            emit_bcast_mul(TOT - 1)
            for tp in range(8):
                emit_proj_half(NCH - 1, tp)

    nc.compile()
    return nc


def _get_nc():
    global _CACHED_NC
    if _CACHED_NC is None:
        _CACHED_NC = _build()
    return _CACHED_NC


def kernel(x, context, context_mask, q_w, q_b, kv_w, kv_b, proj_w, proj_b):
    global LAST_RESULTS
    from concourse.bass_utils import run_bass_kernel_spmd

    x = np.asarray(x, dtype=np.float32)
    context = np.asarray(context, dtype=np.float32)
    context_mask = np.asarray(context_mask)
    q_w = np.asarray(q_w, dtype=np.float32)
    q_b = np.asarray(q_b, dtype=np.float32)
    kv_w = np.asarray(kv_w, dtype=np.float32)
    kv_b = np.asarray(kv_b, dtype=np.float32)
    proj_w = np.asarray(proj_w, dtype=np.float32)
    proj_b = np.asarray(proj_b, dtype=np.float32)

    c = np.ascontiguousarray

    in_maps = []
    for dev in range(8):
        b, g = dev // 2, dev % 2
        gs = g * GF
        m01_np = np.where(context_mask[b], np.float32(0.0), np.float32(1.0))
        h16 = np.float16
        in_maps.append({
            "xT": c(x[b].T.astype(h16)),
            "ctxT": c(context[b].T.astype(h16)),
            "qw": c(q_w[:, gs:gs + GF].astype(h16)),
            "kw": c(kv_w[:, gs:gs + GF].astype(h16)),
            "vw": c(kv_w[:, DIM + gs:DIM + gs + GF].astype(h16)),
            "pw": c(proj_w[gs:gs + GF, :].astype(h16)),
            "qb": c(q_b[gs:gs + GF].reshape(GF // P, P).T),
            "kb": c(kv_b[gs:gs + GF].reshape(GF // P, P).T),
            "vbm": c(m01_np.reshape(S // P, P).T[:, :, None]
                     * kv_b[DIM + gs:DIM + gs + GF][None, None, :]).astype(h16),
            "m01": c(m01_np.reshape(S // P, P).T),
        })

    nc = _get_nc()
    try:
        res = run_bass_kernel_spmd(nc, in_maps, core_ids=list(range(8)))
    except Exception:
        # transient NRT_EXEC_UNIT_UNRECOVERABLE has been observed once on a
        # wedged core; a straight retry recovers it
        res = run_bass_kernel_spmd(nc, in_maps, core_ids=list(range(8)))
    LAST_RESULTS = res

    out = np.empty((B, N, DIM), dtype=np.float32)
    for b in range(B):
        out[b] = (res.results[2 * b]["o"].astype(np.float32)
                  + res.results[2 * b + 1]["o"].astype(np.float32) + proj_b)
    return out
